# revision 1
# baseline (speedup 1.0000x reference)
"""Causal attention head (B=16, S=2048, d=64) on 8 TRN2 NeuronCores.

Data parallel over batch: each core gets 2 batches and computes its full
S x S causal attention.

Per-core algorithm (transposed-scores layout):
  scores_T[j, i] = sum_d k[j,d] q[i,d] / 64      (j on PSUM partitions)
  attn_T = exp(scores_T)  (scores are tiny: |s|<1, so no max-subtraction)
  out[i, :64], l[i] = sum_j attn_T[j, i] * [v[j, :] | 1]   (ones-column trick)
  out[i] /= l[i]
Causality: j-tiles with j > i are never computed; the two diagonal
pair-tiles per i-tile get zeroed via affine_select after exp.
"""

import numpy as np

import concourse.bacc as bacc
import concourse.bass as bass
import concourse.mybir as mybir
import concourse.tile as tile
from concourse.bass_utils import run_bass_kernel_spmd
from concourse.masks import make_identity

F32 = mybir.dt.float32
BF16 = mybir.dt.bfloat16

B, S, D = 16, 2048, 64
N_CORES = 8
BPC = B // N_CORES  # batches per core
P = 128
ITILE = 512               # i-tile width (free dim of scores_T)
N_IT = S // ITILE         # 4 i-tiles
N_JC = S // P             # 16 j-chunks
SCALE = 1.0 / D


import os as _os

QUAD_BUFS = int(_os.environ.get("K_QUAD_BUFS", "2"))
ACC_BUFS = int(_os.environ.get("K_ACC_BUFS", "2"))
TRP_BUFS = int(_os.environ.get("K_TRP_BUFS", "2"))
LAG_N = int(_os.environ.get("K_LAG", "4"))
INJ_N = int(_os.environ.get("K_INJ", "10"))
ATTN_BUFS = int(_os.environ.get("K_ATTN_BUFS", "4"))
NOPACK = int(_os.environ.get("K_NOPACK", "0"))  # timing-only A/B probe
MERGE = int(_os.environ.get("K_MERGE", "1"))  # one pipeline across batches


def build_kernel(loop: int = 0, level: int = 4):
    # level: probe ladder for benchmarking — 1: input DMA only, 2: + stage A,
    # 3: + mm1/exp/mask, 4: full kernel (default; the only correct one),
    # 5: stage A + all matmuls, no ACT/mask, 6: stage A + exp stream only
    nc = bacc.Bacc("TRN2", target_bir_lowering=False, debug=False)
    q_h = nc.dram_tensor("q", [BPC, S, D], F32, kind="ExternalInput").ap()
    k_h = nc.dram_tensor("k", [BPC, S, D], F32, kind="ExternalInput").ap()
    v_h = nc.dram_tensor("v", [BPC, S, D], F32, kind="ExternalInput").ap()
    o_h = nc.dram_tensor("o", [BPC, S, D], F32, kind="ExternalOutput").ap()

    with tile.TileContext(nc) as tc:
        with (
            tc.tile_pool(name="const", bufs=1) as const,
            tc.tile_pool(name="stage", bufs=2) as stage,
            tc.tile_pool(name="qkt", bufs=2) as qkt,
            tc.tile_pool(name="attn", bufs=ATTN_BUFS) as attnp,
            tc.tile_pool(name="outs", bufs=2) as outs,
            tc.tile_pool(name="quad", bufs=QUAD_BUFS, space="PSUM") as quadp,
            tc.tile_pool(name="acc", bufs=ACC_BUFS, space="PSUM") as accp,
            tc.tile_pool(name="trp", bufs=TRP_BUFS, space="PSUM") as trp,
        ):
            ident_f = const.tile([P, P], F32)
            make_identity(nc, ident_f)
            ident_b = const.tile([P, P], BF16)
            nc.vector.tensor_copy(ident_b, ident_f)
            # warm the ACT exp table while the input DMAs run
            warm = const.tile([P, 1], F32)
            nc.scalar.activation(
                warm, ident_f[:, 0:1], mybir.ActivationFunctionType.Exp
            )

            def stage_a_loads(b):
                # ---- stage inputs, natural layout [128, 16, 64]
                # halves so DMA/transpose pipeline at half granularity
                H = N_JC // 2
                qn = stage.tile([P, N_JC, D], F32, tag="qn", name=f"qn{b}")
                kn = stage.tile([P, N_JC, D], F32, tag="kn", name=f"kn{b}")
                vn = stage.tile([P, N_JC, D], F32, tag="vn", name=f"vn{b}")
                vp = stage.tile([P, N_JC, D + 1], BF16, tag="vp", name=f"vp{b}")
                kr = k_h[b].rearrange("(n p) d -> p n d", p=P)
                qr = q_h[b].rearrange("(n p) d -> p n d", p=P)
                vr = v_h[b].rearrange("(n p) d -> p n d", p=P)
                Q = N_JC // 4
                # quarters of k/q interleaved (transposes consume in this
                # order), v halves placed just before their first use
                sched = [
                    (kr, kn, 0), (qr, qn, 0), (kr, kn, 1), (qr, qn, 1),
                    (vr, vn, None), (kr, kn, 2), (qr, qn, 2),
                    (kr, kn, 3), (qr, qn, 3), (vr, vn, None),
                ]
                vh = 0
                for src, dst, qi in sched:
                    if qi is None:
                        sl = slice(H * vh, H * (vh + 1))
                        nc.sync.dma_start(dst[:, sl, :], src[:, sl, :])
                        if level >= 2:
                            nc.gpsimd.tensor_copy(vp[:, sl, 0:D], vn[:, sl, :])
                        vh += 1
                    else:
                        sl = slice(Q * qi, Q * (qi + 1))
                        nc.sync.dma_start(dst[:, sl, :], src[:, sl, :])
                if level >= 2:
                    nc.gpsimd.memset(vp[:, :, D : D + 1], 1.0)
                return qn, kn, vn, vp

            def stage_a_pe(b, qn, kn):
                """Return (qt, kt2, thunks): each thunk emits one PE
                transpose group; caller decides where to interleave them."""
                # K^T interleaved-pairs layout [128, S/2] bf16:
                #  kt2[0:64,  128e+s] = K^T of chunk 2e
                #  kt2[64:128,128e+s] = K^T of chunk 2e+1
                # (transposing a [128, 128] block of TWO adjacent chunks puts
                # chunk 2e on partitions 0:64 and chunk 2e+1 on 64:128).
                # Transposes read the fp32 tiles; bf16 cast is folded into
                # the PSUM->SBUF copy.
                kt2 = qkt.tile([P, S // 2], BF16, tag="kt", name=f"kt{b}")
                qt = qkt.tile([P, S], BF16, tag="qt", name=f"qt{b}")
                thunks = []

                def k_group(g):
                    tr = trp.tile([P, 2 * P], F32, tag="trp", name=f"trk{b}_{g}")
                    for u in range(2):
                        e = 2 * g + u
                        nc.tensor.transpose(
                            tr[:, P * u : P * (u + 1)],
                            kn[:, 2 * e : 2 * e + 2, :],
                            ident_f,
                        )
                    nc.vector.tensor_copy(
                        kt2[:, 2 * P * g : 2 * P * (g + 1)], tr
                    )

                def q_group(g):
                    # Q^T duplicated into both partition halves (two copies
                    # from the same PSUM tile; partition-shifted second copy)
                    tr = trp.tile([P, 4 * P], F32, tag="trp", name=f"trq{b}_{g}")
                    for u in range(4):
                        nc.tensor.transpose(
                            tr[0:D, P * u : P * (u + 1)],
                            qn[:, 4 * g + u, :],
                            ident_f,
                        )
                    sl = slice(4 * P * g, 4 * P * (g + 1))
                    nc.vector.tensor_copy(qt[0:D, sl], tr[0:D])
                    nc.vector.tensor_copy(qt[D : 2 * D, sl], tr[0:D])

                # interleave k/q groups so the data the first matmuls need
                # (low j-chunks, low i-columns) is ready earliest
                for g in range(4):
                    thunks.append(lambda g=g: k_group(g))
                    thunks.append(lambda g=g: q_group(g))
                return qt, kt2, thunks

            def one_pass():
                loaded = [stage_a_loads(b) for b in range(BPC)]
                if level < 2:
                    nc.sync.dma_start(o_h[0, 0:P, :], ident_f[:, 0:D])
                    return
                pe_stage = [
                    stage_a_pe(b, loaded[b][0], loaded[b][1])
                    for b in range(BPC)
                ]
                staged = []
                for b in range(BPC):
                    qt, kt2, thunks = pe_stage[b]
                    staged.append((qt, kt2, loaded[b][3], thunks))
                if level < 3:
                    for _, _, _, thunks in staged:
                        for t in thunks:
                            t()
                    nc.sync.dma_start(o_h[0, 0:P, :], ident_f[:, 0:D])
                    return
                if level == 6:
                    # ACT-throughput probe: same exp stream, psum filled once
                    for b in range(BPC):
                        qt, kt2, vp, thunks = staged[b]
                        for t in thunks:
                            t()
                        st = quadp.tile([P, 2, ITILE], F32, tag="quad")
                        for c in range(2):
                            h = slice(D * c, D * (c + 1))
                            nc.tensor.matmul(
                                st[:, c, :], lhsT=kt2[h, 0:P],
                                rhs=qt[h, 0:ITILE], start=True, stop=True,
                            )
                        for pr in range(20):
                            at = attnp.tile([P, 2, ITILE], BF16, tag="attn")
                            nc.scalar.activation(
                                at, st, mybir.ActivationFunctionType.Exp,
                                scale=SCALE,
                            )
                    nc.sync.dma_start(o_h[0, 0:P, :], ident_f[:, 0:D])
                    return
                LAG = LAG_N  # mm2 trails mm1/exp by LAG pairs: PE (strict FIFO)
                #          must never queue an mm2 whose exp isn't done yet
                pairs = [
                    (it, pr) for it in range(N_IT) for pr in range(2 * (it + 1))
                ]
                INJ_START = INJ_N  # during batch b's main loop, emit batch b+1's
                #                 PE transpose groups starting at this pair

                def stage_c(b, it, out_ps):
                    # normalize + transpose out_ps [65, 512] -> [512, 64]
                    osb = outs.tile([D + 1, ITILE], F32, tag="osb")
                    nc.vector.tensor_copy(osb, out_ps)
                    trq = trp.tile([P, 4, D + 1], F32, tag="trp")
                    for s in range(4):
                        nc.tensor.transpose(
                            trq[:, s, :],
                            osb[:, P * s : P * (s + 1)],
                            ident_f[0 : D + 1, 0 : D + 1],
                        )
                    rec = outs.tile([P, 4], F32, tag="rec")
                    nc.vector.reciprocal(rec, trq[:, :, D])
                    fin = outs.tile([P, 4, D], F32, tag="fin")
                    nc.vector.tensor_tensor(
                        fin,
                        trq[:, :, 0:D],
                        rec[:, :, None].to_broadcast((P, 4, D)),
                        mybir.AluOpType.mult,
                    )
                    r0 = ITILE * it
                    nc.sync.dma_start(
                        o_h[b, r0 : r0 + ITILE, :].rearrange(
                            "(s p) d -> p s d", p=P
                        ),
                        fin,
                    )

                # one software pipeline across BOTH batches: no drain/refill
                # at the batch boundary
                gpairs = [
                    (b, it, pr) for b in range(BPC) for (it, pr) in pairs
                ]
                npb = len(pairs)
                atc = None
                if level == 5:
                    atc = attnp.tile([P, 2, ITILE], BF16, tag="attn")
                    nc.gpsimd.memset(atc, 0.25)
                out_ps_by_key = {}
                at_by_idx = {}

                def do_mm2(idx):
                    b, it, pr = gpairs[idx]
                    vp = staged[b][2]
                    at = at_by_idx.pop(idx)
                    out_ps = out_ps_by_key[(b, it)]
                    npair = 2 * (it + 1)
                    for c in range(2):
                        jc = 2 * pr + c
                        nc.tensor.matmul(
                            out_ps,
                            lhsT=vp[:, jc, :],
                            rhs=at[:, c, :],
                            start=(pr == 0 and c == 0),
                            stop=(pr == npair - 1 and c == 1),
                        )
                    if pr == npair - 1:
                        stage_c(b, it, out_ps_by_key.pop((b, it)))

                for idx, (b, it, pr) in enumerate(gpairs):
                    qt, kt2, vp, _ = staged[b]
                    if b == 0 and pr == 0:
                        # just-in-time: this i-tile needs k_group(it) and
                        # q_group(it); thunks are ordered [k0,q0,k1,q1,...]
                        th = staged[0][3]
                        for _i in range(2):
                            if th:
                                th.pop(0)()
                    if pr == 0:
                        out_ps_by_key[(b, it)] = accp.tile(
                            [D + 1, ITILE], F32, tag="acc", name=f"acc{b}_{it}"
                        )
                    st = quadp.tile([P, 2, ITILE], F32, tag="quad")
                    isl = slice(ITILE * it, ITILE * (it + 1))
                    for c in range(2):
                        # c=0 in PE row group 0, c=1 in row group 64:
                        # the two matmuls stream concurrently
                        h = slice(0, D) if NOPACK else slice(D * c, D * (c + 1))
                        nc.tensor.matmul(
                            st[:, c, :],
                            lhsT=kt2[h, P * pr : P * (pr + 1)],
                            rhs=qt[h, isl],
                            start=True,
                            stop=True,
                        )
                    if level == 5:
                        at_by_idx[idx] = atc
                    else:
                        at = attnp.tile([P, 2, ITILE], BF16, tag="attn")
                        nc.scalar.activation(
                            at, st, mybir.ActivationFunctionType.Exp,
                            scale=SCALE,
                        )
                        if pr >= 2 * it and level != 7:
                            # diagonal pair: zero j > i
                            # keep at[jj,c,ii] iff ii >= 128*(c0+c)+jj
                            c0 = 2 * (pr - 2 * it)
                            nc.gpsimd.affine_select(
                                out=at,
                                in_=at,
                                compare_op=mybir.AluOpType.is_ge,
                                fill=0.0,
                                base=-P * c0,
                                pattern=[[-P, 2], [1, ITILE]],
                                channel_multiplier=-1,
                            )
                        at_by_idx[idx] = at
                    if level == 3:
                        at_by_idx.pop(idx)
                        continue
                    if idx >= LAG and (MERGE or idx % npb >= LAG):
                        do_mm2(idx - LAG)
                    if not MERGE and idx % npb == npb - 1:
                        # flush this batch's lagged mm2s before the next batch
                        for j in range(idx - LAG + 1, idx + 1):
                            do_mm2(j)
                    # inject next batch's transpose groups into this batch's
                    # main loop (PE FIFO: their input DMAs are long done)
                    bi = idx // npb
                    if idx % npb >= INJ_START and bi + 1 < BPC:
                        th = staged[bi + 1][3]
                        if th:
                            th.pop(0)()
                if level != 3 and MERGE:
                    for idx in range(len(gpairs) - LAG, len(gpairs)):
                        do_mm2(idx)
                for b in range(BPC):
                    for t in staged[b][3]:
                        t()
                if level < 4:
                    nc.sync.dma_start(o_h[0, 0:P, :], ident_f[:, 0:D])

            if loop:
                hints = ()
                if _os.environ.get("K_LOOP_HINTS"):
                    hints = (
                        mybir.EngineType.PE,
                        mybir.EngineType.Activation,
                        mybir.EngineType.DVE,
                        mybir.EngineType.Pool,
                        mybir.EngineType.SP,
                    )
                with tc.For_i(0, loop, 1, hint_engines=hints):
                    one_pass()
            else:
                one_pass()

    nc.compile()
    return nc


_CACHE: dict = {}


def _get_nc(loop: int = 0):
    if loop not in _CACHE:
        _CACHE[loop] = build_kernel(loop)
    return _CACHE[loop]


def kernel(q: np.ndarray, k: np.ndarray, v: np.ndarray) -> np.ndarray:
    q = np.ascontiguousarray(q, dtype=np.float32)
    k = np.ascontiguousarray(k, dtype=np.float32)
    v = np.ascontiguousarray(v, dtype=np.float32)
    nc = _get_nc(0)
    in_maps = [
        {
            "q": q[BPC * i : BPC * (i + 1)],
            "k": k[BPC * i : BPC * (i + 1)],
            "v": v[BPC * i : BPC * (i + 1)],
        }
        for i in range(N_CORES)
    ]
    res = run_bass_kernel_spmd(nc, in_maps, list(range(N_CORES)))
    return np.concatenate([res.results[i]["o"] for i in range(N_CORES)], axis=0)

